# revision 79
# baseline (speedup 1.0000x reference)
"""Multi-head attention (B=4, Q=K=2048, D=512, H=8) on 8 TRN2 NeuronCores.

Sharding: every core runs the SAME program but a different (head-pair, q-half)
of every batch: core c owns heads {2*(c%4), 2*(c%4)+1} and query window
[1024*(c//4), 1024*(c//4)+1024) of ALL four batches.  Each batch is truncated
to its OWN KT_b = ceil128(valid_len[b]) -- key positions beyond valid_len have
softmax weight exactly 0, so per-batch truncation is exact and cuts total
attention work from 4*max(KT) to sum(KT).  Every core then processes exactly
sum_b KT_b/128 key-chunks: perfectly balanced by construction.

Device-side choices:
  * Activations transposed ([feature, seq]); matmuls contract the partition dim.
  * Scores computed transposed (S_T[k, q] = K_h @ Q_h^T).  The two heads of a
    core's pair sit on partitions 0-63 / 64-127 of shared q_t/k_t tiles, so
    their C=64 score matmuls land on disjoint PE row-groups (auto
    tile_position (0,0)/(64,0)) and run CONCURRENTLY in the array -- 2x score
    throughput vs. sequential heads.
  * No mask and no exp bias: chunks are either fully valid or the final
    partial chunk, whose invalid key rows are zeroed in v_sb (values AND the
    interleaved ones column), removing them from both the attnV numerator and
    the softmax denominator.  One exp covers both heads' score tiles
    ([128, 1024] PSUM spanning the pair's two banks).
  * Ones-column interleaved into v gives the softmax denominator for free
    (row 64 of each head's [65, 512] attnV accumulator).
  * All inputs arrive pre-packed host-side as [128, N] panels so each tensor
    is ONE large DMA descriptor (16 input DMAs total) -- the sync engine's
    per-descriptor issue cost otherwise starves the front of the kernel.
  * v_sb is a 3D [128, NCH, 130] tile per unit: V-proj runs 4 key-chunks per
    PSUM tile and lands them with two strided 3D casts instead of 8 small
    copies; ones columns are memset once per unit in the prologue.
  * bf16 matmul pipeline with fp32 PSUM; softmax/normalization fp32.
  * Partial-output projection per (core, batch); host sums the 4 head-pair
    partials per (batch, q-half).  Partials in bf16 to halve output DMA.
  * Projections of later units and output-projections of earlier units are
    emitted as filler tasks inside the attention loop so the PE never idles
    while the ACT engine (the attention-phase bottleneck) chews exps.
  * Units run smallest-first (fast DMA-gated start) with the largest third
    (its long ACT-bound window absorbs queued filler work and keeps the PE
    clock warm) and the smallest last (short output tail).  After the final
    exp, output-projection casts/DMAs alternate onto the idle ACT engine and
    second DMA queue.
"""

import functools
from collections import deque

import ml_dtypes
import numpy as np

import concourse.bacc as bacc
import concourse.bass as bass
import concourse.mybir as mybir
from concourse import tile
from concourse.bass_utils import run_bass_kernel_spmd

F32 = mybir.dt.float32
F32R = mybir.dt.float32r
BF16 = mybir.dt.bfloat16

B, Q, KSEQ, D, H = 4, 2048, 2048, 512, 8
DH = D // H          # 64   head dim
QW = 1024            # per-core query window
N_CORES = 8
EXP = mybir.ActivationFunctionType.Exp


@functools.lru_cache(maxsize=4)
def build_nc(kts, vls):
    """One SPMD program; kts/vls are the per-unit (execution-ordered)
    key lengths / valid lens of the 4 batches."""
    assert all(kt % 128 == 0 and 128 <= kt <= KSEQ for kt in kts)
    NCH = [kt // 128 for kt in kts]
    KOFF = np.concatenate([[0], np.cumsum(kts)]).tolist()
    SK = KOFF[-1]

    nc = bacc.Bacc("TRN2", target_bir_lowering=False, debug=False,
                   num_devices=N_CORES)

    def din(name, shape, dt=BF16):
        return nc.dram_tensor(name, shape, dt, kind="ExternalInput").ap()

    xq_d = din("xq_t", [128, 16 * QW])
    xk_d = din("xk_t", [128, 4 * SK])
    xv_d = din("xv_t", [128, 4 * SK])
    wq_d = din("wq_t", [128, D])
    wk_d = din("wk_t", [128, D])
    wv_d = din("wv_t", [128, D])
    wo_d = din("wo_t", [128, D])
    y_d = nc.dram_tensor("y_t", [D, 4 * QW], BF16, kind="ExternalOutput").ap()

    with tile.TileContext(nc) as tc:
        with (
            nc.allow_low_precision(reason="bf16 matmul operands"),
            tc.tile_pool(name="persist", bufs=1) as pp,
            tc.tile_pool(name="cbuf", bufs=1) as cb,
            # 8 PSUM banks: psS 2x[128,1024] score tiles (pair x 512q),
            # psO oA+oB [65,512] attnV accumulators, psA 2x[128,512]
            # projections / denominator broadcast.
            tc.tile_pool(name="psS", bufs=1, space=bass.MemorySpace.PSUM) as psS,
            tc.tile_pool(name="psO", bufs=1, space=bass.MemorySpace.PSUM) as psO,
            tc.tile_pool(name="psA", bufs=2, space=bass.MemorySpace.PSUM) as psA,
        ):
            # ---- persistent tiles ----
            wq = pp.tile([128, D], BF16, tag="wq", name="wq")
            wk = pp.tile([128, D], BF16, tag="wk", name="wk")
            wv = pp.tile([128, D], BF16, tag="wv", name="wv")
            wo = pp.tile([128, D], BF16, tag="wo", name="wo")
            onescr = pp.tile([128, DH], F32, tag="onescr", name="onescr")
            ones_sb = pp.tile([65, DH], F32R, tag="ones", name="ones_sb")
            actwarm = pp.tile([1, 1], F32, tag="actwarm", name="actwarm")

            xq = [pp.tile([128, 4, QW], BF16, tag=f"xq{u}", name=f"xq{u}")
                  for u in range(4)]
            xk = [pp.tile([128, 4, kts[u]], BF16, tag=f"xk{u}", name=f"xk{u}")
                  for u in range(4)]
            xv = [pp.tile([128, 4, kts[u]], BF16, tag=f"xv{u}", name=f"xv{u}")
                  for u in range(4)]
            q_t = [pp.tile([128, QW], BF16, tag=f"q_t{u}", name=f"q_t{u}")
                   for u in range(4)]
            k_t = [pp.tile([128, kts[u]], BF16, tag=f"k_t{u}", name=f"k_t{u}")
                   for u in range(4)]
            v_sb = [pp.tile([128, NCH[u], 130], BF16, tag=f"v{u}",
                            name=f"v{u}") for u in range(4)]

            # ---- DMAs on TWO hardware queues (sync + scalar HWDGE): the
            # k/v stream and the weights/q stream transfer in parallel ----
            nc.sync.dma_start(wk[:], wk_d[:])
            nc.scalar.dma_start(wq[:], wq_d[:])
            # pull the ACT exp table load into the initial DMA wait
            nc.vector.memset(onescr[:], 1.0)
            nc.scalar.activation(actwarm[:], onescr[0:1, 0:1], EXP)
            nc.vector.tensor_copy(ones_sb[64:65, :], onescr[64:65, :])

            # prologue memsets: ones columns for every unit's v_sb; zero the
            # final partial chunk first so its invalid rows stay zero.
            for u in range(4):
                nv = vls[u] - (NCH[u] - 1) * 128
                if nv < 128:
                    if NCH[u] > 1:
                        nc.vector.memset(v_sb[u][:, 0:NCH[u] - 1, 64::65], 1.0)
                    nc.vector.memset(v_sb[u][:, NCH[u] - 1, :], 0.0)
                    nc.vector.memset(v_sb[u][0:nv, NCH[u] - 1, 64::65], 1.0)
                else:
                    nc.vector.memset(v_sb[u][:, :, 64::65], 1.0)
            nc.sync.dma_start(xk[0][:], xk_d[:, 4 * KOFF[0]:4 * KOFF[1]])
            nc.scalar.dma_start(wv[:], wv_d[:])
            nc.sync.dma_start(xv[0][:], xv_d[:, 4 * KOFF[0]:4 * KOFF[1]])
            nc.scalar.dma_start(xq[0][:], xq_d[:, 0:4 * QW])
            nc.scalar.dma_start(wo[:], wo_d[:])
            for u in range(1, 4):
                nc.sync.dma_start(xk[u][:],
                                  xk_d[:, 4 * KOFF[u]:4 * KOFF[u + 1]])
                nc.sync.dma_start(xv[u][:],
                                  xv_d[:, 4 * KOFF[u]:4 * KOFF[u + 1]])
                nc.scalar.dma_start(xq[u][:],
                                    xq_d[:, u * 4 * QW:(u + 1) * 4 * QW])

            # ---- projection / output-projection task factories ----
            def qproj(u, qs):
                def run():
                    ps = psA.tile([128, 512], F32, tag="proj", name="ps")
                    for ic in range(4):
                        nc.tensor.matmul(
                            ps[:], wq[:, ic * 128:(ic + 1) * 128],
                            xq[u][:, ic, qs * 512:(qs + 1) * 512],
                            start=(ic == 0), stop=(ic == 3))
                    nc.vector.tensor_copy(q_t[u][:, qs * 512:(qs + 1) * 512],
                                          ps[:])
                return run

            def kproj(u, s, w):
                def run():
                    ps = psA.tile([128, 512], F32, tag="proj", name="ps")
                    for ic in range(4):
                        nc.tensor.matmul(ps[:, :w],
                                         wk[:, ic * 128:(ic + 1) * 128],
                                         xk[u][:, ic, s:s + w],
                                         start=(ic == 0), stop=(ic == 3))
                    nc.vector.tensor_copy(k_t[u][:, s:s + w], ps[:, :w])
                return run

            def vproj(u, g):
                # one group = up to 4 key-chunks through a [128, 4, 128] PSUM
                # tile, landed with two strided 3D casts per head
                kcs = list(range(g * 4, min(NCH[u], g * 4 + 4)))
                nfull = sum(1 for kc in kcs if vls[u] - kc * 128 >= 128)

                def run():
                    ps = psA.tile([128, 4, 128], F32, tag="proj", name="ps")
                    for j, kc in enumerate(kcs):
                        for ic in range(4):
                            nc.tensor.matmul(
                                ps[:, j, :],
                                xv[u][:, ic, kc * 128:(kc + 1) * 128],
                                wv[:, ic * 128:(ic + 1) * 128],
                                start=(ic == 0), stop=(ic == 3))
                    for h in range(2):
                        if nfull:
                            nc.vector.tensor_copy(
                                v_sb[u][:, kcs[0]:kcs[0] + nfull,
                                        h * 65:h * 65 + 64],
                                ps[:, 0:nfull, h * 64:(h + 1) * 64])
                    for j, kc in enumerate(kcs[nfull:], start=nfull):
                        nv = vls[u] - kc * 128
                        for h in range(2):
                            nc.vector.tensor_copy(
                                v_sb[u][0:nv, kc, h * 65:h * 65 + 64],
                                ps[0:nv, j, h * 64:(h + 1) * 64])
                return run

            tail = {"on": False, "flip": 0}

            def oproj(u, ot, qs):
                def run():
                    # after the last exp the psS score banks are free: rotate
                    # tail output projections over them for deeper pipelining
                    if tail["on"]:
                        ps = psS.tile([128, 512], F32, tag="s", name="ps")
                    else:
                        ps = psA.tile([128, 512], F32, tag="proj", name="ps")
                    nc.tensor.matmul(ps[:], wo[:, ot * 128:(ot + 1) * 128],
                                     o_pair[u][:, qs * 512:(qs + 1) * 512],
                                     start=True, stop=True)
                    y_sb = cb.tile([128, 512], BF16, tag="y", bufs=2,
                                   name="y_sb")
                    # after the last exp the ACT engine and second DMA queue
                    # are idle: split the output tail across both engine pairs
                    tail["flip"] ^= 1
                    if tail["on"] and tail["flip"]:
                        nc.scalar.copy(y_sb[:], ps[:])
                        dmae = nc.scalar
                    else:
                        nc.vector.tensor_copy(y_sb[:], ps[:])
                        dmae = nc.sync
                    dmae.dma_start(
                        y_d[ot * 128:(ot + 1) * 128,
                            u * QW + qs * 512:u * QW + (qs + 1) * 512],
                        y_sb[:])
                return run

            def proj_tasks(u):
                # each task carries the first local attention step (ip*NCH+kt)
                # that consumes its output, enabling just-in-time draining
                t = []
                for s in range(0, kts[u], 512):
                    t.append(("proj", u, s // 128,
                              kproj(u, s, min(512, kts[u] - s))))
                for g in range((NCH[u] + 3) // 4):
                    t.append(("proj", u, g * 4, vproj(u, g)))
                t.append(("proj", u, 0, qproj(u, 0)))
                t.append(("proj", u, NCH[u], qproj(u, 1)))
                t.sort(key=lambda x: x[2])
                return t

            o_pair = {}

            # ---- flat attention pipeline over (unit, ip, kt) steps ----
            fillers = deque()
            # unit 0: only what the first attention steps need goes inline;
            # the rest becomes fillers drained just in time
            for f in proj_tasks(0):
                if f[2] == 0:
                    f[3]()
                else:
                    fillers.append(f)
            fillers.extend(proj_tasks(1))

            steps = [(u, ip, kt) for u in range(4) for ip in range(2)
                     for kt in range(NCH[u])]

            # ONE [128, 2048] score tile (same 4 banks as 2x[128,1024]):
            # chunk kt writes half kt%2, and a single exp covers a chunk PAIR
            # -- (2048+352)/1.2 = 2.0us vs 2x1.147us, cutting the ACT floor
            # from 55.1us to 48.6us.  WAR is unchanged: scores kt+2 waits for
            # the exp that reads its half, exactly as with two buffers.
            s_big = psS.tile([128, 2048], F32, tag="s", name="s_big")

            def scores(u, ip, kt):
                half = (kt % 2) * 1024
                for hf in range(2):
                    nc.tensor.matmul(
                        s_big[:, half + hf * 512:half + (hf + 1) * 512],
                        k_t[u][hf * 64:(hf + 1) * 64, kt * 128:(kt + 1) * 128],
                        q_t[u][hf * 64:(hf + 1) * 64,
                               ip * 512:(ip + 1) * 512],
                        start=True, stop=True)

            def exp_batch(n):
                p_sb = cb.tile([128, 2048], BF16, tag="p", bufs=3, name="p_sb")
                nc.scalar.activation(p_sb[:, 0:n * 1024],
                                     s_big[:, 0:n * 1024], EXP, scale=1.0)
                return p_sb

            acc = {}

            def attnv(u, ip, chunks, p_sb):
                for kt in chunks:
                    if kt == 0:
                        acc["oA"] = psO.tile([65, 512], F32, tag="oA",
                                             name="oA")
                        acc["oB"] = psO.tile([65, 512], F32, tag="oB",
                                             name="oB")
                    half = (kt % 2) * 1024
                    for hf, o_ps in enumerate((acc["oA"], acc["oB"])):
                        nc.tensor.matmul(
                            o_ps[:], v_sb[u][:, kt, hf * 65:hf * 65 + 65],
                            p_sb[:, half + hf * 512:half + (hf + 1) * 512],
                            start=(kt == 0), stop=(kt == NCH[u] - 1))

            def normalize(u, ip):
                if u not in o_pair:
                    o_pair[u] = cb.tile([128, QW], BF16, tag="o", bufs=2,
                                        name="o_pair")
                cols = slice(ip * 512, (ip + 1) * 512)
                for hf, o_ps in enumerate((acc["oA"], acc["oB"])):
                    dn = cb.tile([65, 512], F32R, tag="dn", bufs=2, name="dn")
                    nc.vector.tensor_copy(dn[64:65, :], o_ps[64:65, :])
                    bc_ps = psA.tile([64, 512], F32, tag="proj", name="bc_ps")
                    nc.tensor.matmul(bc_ps[:], ones_sb[64:65, :],
                                     dn[64:65, :], start=True, stop=True)
                    inv_sb = cb.tile([64, 512], F32, tag="invb", bufs=2,
                                     name="inv_sb")
                    nc.vector.reciprocal_approx_fast(inv_sb[:], bc_ps[:])
                    if hf == 0:
                        nc.vector.tensor_mul(o_pair[u][0:64, cols],
                                             o_ps[0:64, :], inv_sb[:])
                    else:
                        # DVE lanes can't cross partitions: normalize into a
                        # scratch tile, DMA-hop to partitions 64-127
                        o_tmp = cb.tile([64, 512], BF16, tag="otmp", bufs=2,
                                        name="o_tmp")
                        nc.vector.tensor_mul(o_tmp[:], o_ps[0:64, :],
                                             inv_sb[:])
                        he = nc.scalar if tail["on"] else nc.sync
                        he.dma_start(o_pair[u][64:128, cols], o_tmp[:])

            prev = None
            pend = []
            for idx, step in enumerate(steps):
                u, ip, kt = step
                # just-in-time drain: run any projection task whose output an
                # imminent step consumes (in-order PE queue => emitting it
                # after its consumer would deadlock); leave the rest queued.
                # The horizon spans unit boundaries so the next unit's first
                # projections land before its first scores.
                horizon = steps[idx:idx + 4]

                def is_due(t):
                    if t[0] != "proj":
                        return False
                    for hu, hip, hkt in horizon:
                        if t[1] < hu or (t[1] == hu and
                                         t[2] <= hip * NCH[hu] + hkt + 1):
                            return True
                    return False

                due = [t for t in fillers if is_due(t)]
                if due:
                    rest = [t for t in fillers if t not in due]
                    fillers.clear()
                    fillers.extend(rest)
                    for t in due:
                        t[3]()
                scores(u, ip, kt)
                pend.append(kt)
                if kt % 2 == 1 or kt == NCH[u] - 1:
                    batch = (u, ip, list(pend), exp_batch(len(pend)))
                    pend = []
                else:
                    batch = None
                if batch is not None and prev is not None:
                    pu, pip, pchunks, pp_ = prev
                    attnv(pu, pip, pchunks, pp_)
                    if pchunks[-1] == NCH[pu] - 1:
                        normalize(pu, pip)
                        # the q-half's output projection unblocks as soon as
                        # its normalize lands -- don't wait for ip=1
                        for ot in range(4):
                            fillers.append(("oproj", pu, 0,
                                            oproj(pu, ot, pip)))
                        if pip == 1 and pu + 2 <= 3:
                            # interleave the new unit's projections among the
                            # queued oproj tasks so each attention step can
                            # retire one of each kind
                            from itertools import zip_longest
                            old = list(fillers)
                            fillers.clear()
                            for pair in zip_longest(proj_tasks(pu + 2), old):
                                for t in pair:
                                    if t is not None:
                                        fillers.append(t)
                # near an ip boundary keep the DVE queue clear so the
                # normalize chain (dn copy -> bcast -> recip -> mul) that
                # gates the next ip's attnv isn't stuck behind filler casts
                if kt < NCH[u] - 2:
                    if fillers:
                        fillers.popleft()[3]()
                    extra = 3 if u == 3 else 1
                    while extra and fillers and fillers[0][0] == "oproj":
                        fillers.popleft()[3]()
                        extra -= 1
                if batch is not None:
                    prev = batch
            pu, pip, pchunks, pp_ = prev
            attnv(pu, pip, pchunks, pp_)
            tail["on"] = True
            normalize(pu, pip)
            while fillers:
                fillers.popleft()[3]()
            for ot in range(4):
                oproj(pu, ot, pip)()

    nc.compile()
    return nc


def ceil128(v):
    return int(min(KSEQ, max(128, ((int(v) + 127) // 128) * 128)))


def plan(valid_lens):
    vl = np.asarray(valid_lens).astype(np.int64)
    asc = sorted(range(B), key=lambda b: (ceil128(vl[b]), b))
    # small units first (fast DMA-gated start), the giant third (its long
    # ACT-bound window absorbs earlier output projections and keeps the PE
    # warm), the smallest last (short output tail)
    order = [asc[1], asc[2], asc[3], asc[0]]
    kts = tuple(ceil128(vl[b]) for b in order)
    vls = tuple(int(min(vl[b], kts[i])) for i, b in enumerate(order))
    return order, kts, vls


def _pack4(x):
    """[512, N] -> [128, 4N] with the four 128-row blocks side by side."""
    n = x.shape[1]
    return x.reshape(4, 128, n).transpose(1, 0, 2).reshape(128, 4 * n)


def make_in_maps(queries, keys, values, valid_lens, W_q, W_k, W_v, W_o,
                 order, kts):
    bf = ml_dtypes.bfloat16
    queries = np.asarray(queries, np.float32)
    keys = np.asarray(keys, np.float32)
    values = np.asarray(values, np.float32)
    W_q = np.asarray(W_q, np.float32)
    W_k = np.asarray(W_k, np.float32)
    W_v = np.asarray(W_v, np.float32)
    W_o = np.asarray(W_o, np.float32)
    in_maps = []
    for c in range(N_CORES):
        hp, qp = c % 4, c // 4
        fsl = slice(hp * 128, (hp + 1) * 128)
        qsl = slice(qp * QW, (qp + 1) * QW)
        xq = np.concatenate([_pack4(queries[b, qsl, :].T) for b in order],
                            axis=1)
        xk = np.concatenate([_pack4(keys[b, :kts[i], :].T)
                             for i, b in enumerate(order)], axis=1)
        xv = np.concatenate([_pack4(values[b, :kts[i], :].T)
                             for i, b in enumerate(order)], axis=1)
        in_maps.append({
            "xq_t": np.ascontiguousarray(xq).astype(bf),
            "xk_t": np.ascontiguousarray(xk).astype(bf),
            "xv_t": np.ascontiguousarray(xv).astype(bf),
            "wq_t": np.ascontiguousarray(_pack4((W_q[fsl, :] / 8.0).T)).astype(bf),
            "wk_t": np.ascontiguousarray(_pack4(W_k[fsl, :].T)).astype(bf),
            "wv_t": np.ascontiguousarray(_pack4(W_v[fsl, :].T)).astype(bf),
            "wo_t": np.ascontiguousarray(W_o[:, fsl].T).astype(bf),
        })
    return in_maps


def assemble(results, order):
    out = np.empty((B, Q, D), np.float32)
    for qp in range(2):
        for i, b in enumerate(order):
            acc = np.zeros((D, QW), np.float32)
            for hp in range(4):
                acc += np.asarray(results[qp * 4 + hp]["y_t"]
                                  [:, i * QW:(i + 1) * QW], np.float32)
            out[b, qp * QW:(qp + 1) * QW, :] = acc.T
    return out


def kernel(queries, keys, values, valid_lens, W_q, W_k, W_v, W_o):
    order, kts, vls = plan(valid_lens)
    nc = build_nc(kts, vls)
    in_maps = make_in_maps(queries, keys, values, valid_lens,
                           W_q, W_k, W_v, W_o, order, kts)
    res = run_bass_kernel_spmd(nc, in_maps, list(range(N_CORES))).results
    return assemble(res, order)


# revision 81
# speedup vs baseline: 1.0731x; 1.0731x over previous
"""Multi-head attention (B=4, Q=K=2048, D=512, H=8) on 8 TRN2 NeuronCores.

Sharding: every core runs the SAME program but a different (head-pair, q-half)
of every batch: core c owns heads {2*(c%4), 2*(c%4)+1} and query window
[1024*(c//4), 1024*(c//4)+1024) of ALL four batches.  Each batch is truncated
to its OWN KT_b = ceil128(valid_len[b]) -- key positions beyond valid_len have
softmax weight exactly 0, so per-batch truncation is exact and cuts total
attention work from 4*max(KT) to sum(KT).  Every core then processes exactly
sum_b KT_b/128 key-chunks: perfectly balanced by construction.

Device-side choices:
  * Activations transposed ([feature, seq]); matmuls contract the partition dim.
  * Scores computed transposed (S_T[k, q] = K_h @ Q_h^T).  The two heads of a
    core's pair sit on partitions 0-63 / 64-127 of shared q_t/k_t tiles, so
    their C=64 score matmuls land on disjoint PE row-groups (auto
    tile_position (0,0)/(64,0)) and run CONCURRENTLY in the array -- 2x score
    throughput vs. sequential heads.
  * No mask and no exp bias: chunks are either fully valid or the final
    partial chunk, whose invalid key rows are zeroed in v_sb (values AND the
    interleaved ones column), removing them from both the attnV numerator and
    the softmax denominator.  One exp covers both heads' score tiles
    ([128, 1024] PSUM spanning the pair's two banks).
  * Ones-column interleaved into v gives the softmax denominator for free
    (row 64 of each head's [65, 512] attnV accumulator).
  * All inputs arrive pre-packed host-side as [128, N] panels so each tensor
    is ONE large DMA descriptor (16 input DMAs total) -- the sync engine's
    per-descriptor issue cost otherwise starves the front of the kernel.
  * v_sb is a 3D [128, NCH, 130] tile per unit: V-proj runs 4 key-chunks per
    PSUM tile and lands them with two strided 3D casts instead of 8 small
    copies; ones columns are memset once per unit in the prologue.
  * bf16 matmul pipeline with fp32 PSUM; softmax/normalization fp32.
  * Partial-output projection per (core, batch); host sums the 4 head-pair
    partials per (batch, q-half).  Partials in bf16 to halve output DMA.
  * Projections of later units and output-projections of earlier units are
    emitted as filler tasks inside the attention loop so the PE never idles
    while the ACT engine (the attention-phase bottleneck) chews exps.
  * Units run smallest-first (fast DMA-gated start) with the largest third
    (its long ACT-bound window absorbs queued filler work and keeps the PE
    clock warm) and the smallest last (short output tail).  After the final
    exp, output-projection casts/DMAs alternate onto the idle ACT engine and
    second DMA queue.
"""

import functools
from collections import deque

import ml_dtypes
import numpy as np

import concourse.bacc as bacc
import concourse.bass as bass
import concourse.mybir as mybir
from concourse import tile
from concourse.bass_utils import run_bass_kernel_spmd

F32 = mybir.dt.float32
F32R = mybir.dt.float32r
BF16 = mybir.dt.bfloat16

B, Q, KSEQ, D, H = 4, 2048, 2048, 512, 8
DH = D // H          # 64   head dim
QW = 1024            # per-core query window
N_CORES = 8
EXP = mybir.ActivationFunctionType.Exp


@functools.lru_cache(maxsize=4)
def build_nc(kts, vls):
    """One SPMD program; kts/vls are the per-unit (execution-ordered)
    key lengths / valid lens of the 4 batches."""
    assert all(kt % 128 == 0 and 128 <= kt <= KSEQ for kt in kts)
    NCH = [kt // 128 for kt in kts]
    KOFF = np.concatenate([[0], np.cumsum(kts)]).tolist()
    SK = KOFF[-1]

    nc = bacc.Bacc("TRN2", target_bir_lowering=False, debug=False,
                   num_devices=N_CORES)

    def din(name, shape, dt=BF16):
        return nc.dram_tensor(name, shape, dt, kind="ExternalInput").ap()

    xq_d = din("xq_t", [128, 16 * QW])
    xk_d = din("xk_t", [128, 4 * SK])
    xv_d = din("xv_t", [128, 4 * SK])
    wq_d = din("wq_t", [128, D])
    wk_d = din("wk_t", [128, D])
    wv_d = din("wv_t", [128, D])
    wo_d = din("wo_t", [128, D])
    y_d = nc.dram_tensor("y_t", [D, 4 * QW], BF16, kind="ExternalOutput").ap()

    with tile.TileContext(nc) as tc:
        with (
            nc.allow_low_precision(reason="bf16 matmul operands"),
            tc.tile_pool(name="persist", bufs=1) as pp,
            tc.tile_pool(name="cbuf", bufs=1) as cb,
            # 8 PSUM banks: psS 2x[128,1024] score tiles (pair x 512q),
            # psO oA+oB [65,512] attnV accumulators, psA 2x[128,512]
            # projections / denominator broadcast.
            tc.tile_pool(name="psS", bufs=2, space=bass.MemorySpace.PSUM) as psS,
            tc.tile_pool(name="psO", bufs=1, space=bass.MemorySpace.PSUM) as psO,
            tc.tile_pool(name="psA", bufs=2, space=bass.MemorySpace.PSUM) as psA,
        ):
            # ---- persistent tiles ----
            wq = pp.tile([128, D], BF16, tag="wq", name="wq")
            wk = pp.tile([128, D], BF16, tag="wk", name="wk")
            wv = pp.tile([128, D], BF16, tag="wv", name="wv")
            wo = pp.tile([128, D], BF16, tag="wo", name="wo")
            onescr = pp.tile([128, DH], F32, tag="onescr", name="onescr")
            ones_sb = pp.tile([65, DH], F32R, tag="ones", name="ones_sb")
            actwarm = pp.tile([1, 1], F32, tag="actwarm", name="actwarm")

            xq = [pp.tile([128, 4, QW], BF16, tag=f"xq{u}", name=f"xq{u}")
                  for u in range(4)]
            xk = [pp.tile([128, 4, kts[u]], BF16, tag=f"xk{u}", name=f"xk{u}")
                  for u in range(4)]
            xv = [pp.tile([128, 4, kts[u]], BF16, tag=f"xv{u}", name=f"xv{u}")
                  for u in range(4)]
            q_t = [pp.tile([128, QW], BF16, tag=f"q_t{u}", name=f"q_t{u}")
                   for u in range(4)]
            k_t = [pp.tile([128, kts[u]], BF16, tag=f"k_t{u}", name=f"k_t{u}")
                   for u in range(4)]
            v_sb = [pp.tile([128, NCH[u], 130], BF16, tag=f"v{u}",
                            name=f"v{u}") for u in range(4)]

            # ---- DMAs on TWO hardware queues (sync + scalar HWDGE): the
            # k/v stream and the weights/q stream transfer in parallel ----
            nc.sync.dma_start(wk[:], wk_d[:])
            nc.scalar.dma_start(wq[:], wq_d[:])
            # pull the ACT exp table load into the initial DMA wait
            nc.vector.memset(onescr[:], 1.0)
            nc.scalar.activation(actwarm[:], onescr[0:1, 0:1], EXP)
            nc.vector.tensor_copy(ones_sb[64:65, :], onescr[64:65, :])

            # prologue memsets: ones columns for every unit's v_sb; zero the
            # final partial chunk first so its invalid rows stay zero.
            for u in range(4):
                nv = vls[u] - (NCH[u] - 1) * 128
                if nv < 128:
                    if NCH[u] > 1:
                        nc.vector.memset(v_sb[u][:, 0:NCH[u] - 1, 64::65], 1.0)
                    nc.vector.memset(v_sb[u][:, NCH[u] - 1, :], 0.0)
                    nc.vector.memset(v_sb[u][0:nv, NCH[u] - 1, 64::65], 1.0)
                else:
                    nc.vector.memset(v_sb[u][:, :, 64::65], 1.0)
            nc.sync.dma_start(xk[0][:], xk_d[:, 4 * KOFF[0]:4 * KOFF[1]])
            nc.scalar.dma_start(wv[:], wv_d[:])
            nc.sync.dma_start(xv[0][:], xv_d[:, 4 * KOFF[0]:4 * KOFF[1]])
            nc.scalar.dma_start(xq[0][:], xq_d[:, 0:4 * QW])
            nc.scalar.dma_start(wo[:], wo_d[:])
            for u in range(1, 4):
                nc.sync.dma_start(xk[u][:],
                                  xk_d[:, 4 * KOFF[u]:4 * KOFF[u + 1]])
                nc.sync.dma_start(xv[u][:],
                                  xv_d[:, 4 * KOFF[u]:4 * KOFF[u + 1]])
                nc.scalar.dma_start(xq[u][:],
                                    xq_d[:, u * 4 * QW:(u + 1) * 4 * QW])

            # ---- projection / output-projection task factories ----
            def qproj(u, qs):
                def run():
                    ps = psA.tile([128, 512], F32, tag="proj", name="ps")
                    for ic in range(4):
                        nc.tensor.matmul(
                            ps[:], wq[:, ic * 128:(ic + 1) * 128],
                            xq[u][:, ic, qs * 512:(qs + 1) * 512],
                            start=(ic == 0), stop=(ic == 3))
                    nc.vector.tensor_copy(q_t[u][:, qs * 512:(qs + 1) * 512],
                                          ps[:])
                return run

            def kproj(u, s, w):
                def run():
                    ps = psA.tile([128, 512], F32, tag="proj", name="ps")
                    for ic in range(4):
                        nc.tensor.matmul(ps[:, :w],
                                         wk[:, ic * 128:(ic + 1) * 128],
                                         xk[u][:, ic, s:s + w],
                                         start=(ic == 0), stop=(ic == 3))
                    nc.vector.tensor_copy(k_t[u][:, s:s + w], ps[:, :w])
                return run

            def vproj(u, g):
                # one group = up to 4 key-chunks through a [128, 4, 128] PSUM
                # tile, landed with two strided 3D casts per head
                kcs = list(range(g * 4, min(NCH[u], g * 4 + 4)))
                nfull = sum(1 for kc in kcs if vls[u] - kc * 128 >= 128)

                def run():
                    ps = psA.tile([128, 4, 128], F32, tag="proj", name="ps")
                    for j, kc in enumerate(kcs):
                        for ic in range(4):
                            nc.tensor.matmul(
                                ps[:, j, :],
                                xv[u][:, ic, kc * 128:(kc + 1) * 128],
                                wv[:, ic * 128:(ic + 1) * 128],
                                start=(ic == 0), stop=(ic == 3))
                    for h in range(2):
                        if nfull:
                            nc.vector.tensor_copy(
                                v_sb[u][:, kcs[0]:kcs[0] + nfull,
                                        h * 65:h * 65 + 64],
                                ps[:, 0:nfull, h * 64:(h + 1) * 64])
                    for j, kc in enumerate(kcs[nfull:], start=nfull):
                        nv = vls[u] - kc * 128
                        for h in range(2):
                            nc.vector.tensor_copy(
                                v_sb[u][0:nv, kc, h * 65:h * 65 + 64],
                                ps[0:nv, j, h * 64:(h + 1) * 64])
                return run

            tail = {"on": False, "flip": 0}

            def oproj(u, ot, qs):
                def run():
                    # after the last exp the psS score banks are free: rotate
                    # tail output projections over them for deeper pipelining
                    if tail["on"]:
                        ps = psS.tile([128, 512], F32, tag="s", name="ps")
                    else:
                        ps = psA.tile([128, 512], F32, tag="proj", name="ps")
                    nc.tensor.matmul(ps[:], wo[:, ot * 128:(ot + 1) * 128],
                                     o_pair[u][:, qs * 512:(qs + 1) * 512],
                                     start=True, stop=True)
                    y_sb = cb.tile([128, 512], BF16, tag="y", bufs=2,
                                   name="y_sb")
                    # after the last exp the ACT engine and second DMA queue
                    # are idle: split the output tail across both engine pairs
                    tail["flip"] ^= 1
                    if tail["on"] and tail["flip"]:
                        nc.scalar.copy(y_sb[:], ps[:])
                        dmae = nc.scalar
                    else:
                        nc.vector.tensor_copy(y_sb[:], ps[:])
                        dmae = nc.sync
                    dmae.dma_start(
                        y_d[ot * 128:(ot + 1) * 128,
                            u * QW + qs * 512:u * QW + (qs + 1) * 512],
                        y_sb[:])
                return run

            def proj_tasks(u):
                # each task carries the first local attention step (ip*NCH+kt)
                # that consumes its output, enabling just-in-time draining
                t = []
                for s in range(0, kts[u], 512):
                    t.append(("proj", u, s // 128,
                              kproj(u, s, min(512, kts[u] - s))))
                for g in range((NCH[u] + 3) // 4):
                    t.append(("proj", u, g * 4, vproj(u, g)))
                t.append(("proj", u, 0, qproj(u, 0)))
                t.append(("proj", u, NCH[u], qproj(u, 1)))
                t.sort(key=lambda x: x[2])
                return t

            o_pair = {}

            # ---- flat attention pipeline over (unit, ip, kt) steps ----
            fillers = deque()
            # unit 0: only what the first attention steps need goes inline;
            # the rest becomes fillers drained just in time
            for f in proj_tasks(0):
                if f[2] == 0:
                    f[3]()
                else:
                    fillers.append(f)
            fillers.extend(proj_tasks(1))

            # (u, ip) blocks are self-contained (accumulators alloc at
            # kt=0, normalize at block end), so interleave the tiny last
            # unit's q-halves between the giant's: the giant's long ip1
            # window absorbs the output work that otherwise lands in a
            # serialized post-giant dead zone, and the final tail is just
            # the last 2-step block
            blocks = [(0, 0), (0, 1), (1, 0), (1, 1),
                      (2, 0), (3, 0), (2, 1), (3, 1)]
            steps = [(u, ip, kt) for u, ip in blocks
                     for kt in range(NCH[u])]

            def scores(u, ip, kt):
                s_ps = psS.tile([128, 1024], F32, tag="s", name="s_ps")
                for hf in range(2):
                    nc.tensor.matmul(
                        s_ps[:, hf * 512:(hf + 1) * 512],
                        k_t[u][hf * 64:(hf + 1) * 64, kt * 128:(kt + 1) * 128],
                        q_t[u][hf * 64:(hf + 1) * 64,
                               ip * 512:(ip + 1) * 512],
                        start=True, stop=True)
                p_sb = cb.tile([128, 1024], BF16, tag="p", bufs=4, name="p_sb")
                nc.scalar.activation(p_sb[:], s_ps[:], EXP, scale=1.0)
                return p_sb

            acc = {}

            def attnv(u, ip, kt, p_sb):
                if kt == 0:
                    acc["oA"] = psO.tile([65, 512], F32, tag="oA", name="oA")
                    acc["oB"] = psO.tile([65, 512], F32, tag="oB", name="oB")
                for hf, o_ps in enumerate((acc["oA"], acc["oB"])):
                    nc.tensor.matmul(
                        o_ps[:], v_sb[u][:, kt, hf * 65:hf * 65 + 65],
                        p_sb[:, hf * 512:(hf + 1) * 512],
                        start=(kt == 0), stop=(kt == NCH[u] - 1))

            def normalize(u, ip):
                if u not in o_pair:
                    o_pair[u] = cb.tile([128, QW], BF16, tag="o", bufs=2,
                                        name="o_pair")
                cols = slice(ip * 512, (ip + 1) * 512)
                for hf, o_ps in enumerate((acc["oA"], acc["oB"])):
                    dn = cb.tile([65, 512], F32R, tag="dn", bufs=2, name="dn")
                    nc.vector.tensor_copy(dn[64:65, :], o_ps[64:65, :])
                    bc_ps = psA.tile([64, 512], F32, tag="proj", name="bc_ps")
                    nc.tensor.matmul(bc_ps[:], ones_sb[64:65, :],
                                     dn[64:65, :], start=True, stop=True)
                    inv_sb = cb.tile([64, 512], F32, tag="invb", bufs=2,
                                     name="inv_sb")
                    nc.vector.reciprocal_approx_fast(inv_sb[:], bc_ps[:])
                    if hf == 0:
                        nc.vector.tensor_mul(o_pair[u][0:64, cols],
                                             o_ps[0:64, :], inv_sb[:])
                    else:
                        # DVE lanes can't cross partitions: normalize into a
                        # scratch tile, DMA-hop to partitions 64-127
                        o_tmp = cb.tile([64, 512], BF16, tag="otmp", bufs=2,
                                        name="o_tmp")
                        nc.vector.tensor_mul(o_tmp[:], o_ps[0:64, :],
                                             inv_sb[:])
                        he = nc.scalar if tail["on"] else nc.sync
                        he.dma_start(o_pair[u][64:128, cols], o_tmp[:])

            prev = None
            for idx, step in enumerate(steps):
                u, ip, kt = step
                # just-in-time drain: run any projection task whose output an
                # imminent step consumes (in-order PE queue => emitting it
                # after its consumer would deadlock); leave the rest queued.
                # The horizon spans unit boundaries so the next unit's first
                # projections land before its first scores.
                horizon = steps[idx:idx + 4]

                def is_due(t):
                    if t[0] != "proj":
                        return False
                    for hu, hip, hkt in horizon:
                        if t[1] < hu or (t[1] == hu and
                                         t[2] <= hip * NCH[hu] + hkt + 1):
                            return True
                    return False

                due = [t for t in fillers if is_due(t)]
                if due:
                    rest = [t for t in fillers if t not in due]
                    fillers.clear()
                    fillers.extend(rest)
                    for t in due:
                        t[3]()
                p = scores(u, ip, kt)
                if prev is not None:
                    pu, pip, pkt = prev[0]
                    attnv(pu, pip, pkt, prev[1])
                    if pkt == NCH[pu] - 1:
                        normalize(pu, pip)
                        # the q-half's output projection unblocks as soon as
                        # its normalize lands -- don't wait for ip=1
                        for ot in range(4):
                            fillers.append(("oproj", pu, 0,
                                            oproj(pu, ot, pip)))
                        if pip == 1 and pu + 2 <= 3:
                            # interleave the new unit's projections among the
                            # queued oproj tasks so each attention step can
                            # retire one of each kind
                            from itertools import zip_longest
                            old = list(fillers)
                            fillers.clear()
                            for pair in zip_longest(proj_tasks(pu + 2), old):
                                for t in pair:
                                    if t is not None:
                                        fillers.append(t)
                # near an ip boundary keep the DVE queue clear so the
                # normalize chain (dn copy -> bcast -> recip -> mul) that
                # gates the next ip's attnv isn't stuck behind filler casts
                if kt < NCH[u] - 2:
                    if fillers:
                        fillers.popleft()[3]()
                    extra = 3 if u == 3 else 1
                    while extra and fillers and fillers[0][0] == "oproj":
                        fillers.popleft()[3]()
                        extra -= 1
                prev = (step, p)
            pu, pip, pkt = prev[0]
            attnv(pu, pip, pkt, prev[1])
            tail["on"] = True
            normalize(pu, pip)
            while fillers:
                fillers.popleft()[3]()
            for ot in range(4):
                oproj(pu, ot, pip)()

    nc.compile()
    return nc


def ceil128(v):
    return int(min(KSEQ, max(128, ((int(v) + 127) // 128) * 128)))


def plan(valid_lens):
    vl = np.asarray(valid_lens).astype(np.int64)
    asc = sorted(range(B), key=lambda b: (ceil128(vl[b]), b))
    # small units first (fast DMA-gated start), the giant third (its long
    # ACT-bound window absorbs earlier output projections and keeps the PE
    # warm), the smallest last (short output tail)
    order = [asc[1], asc[2], asc[3], asc[0]]
    kts = tuple(ceil128(vl[b]) for b in order)
    vls = tuple(int(min(vl[b], kts[i])) for i, b in enumerate(order))
    return order, kts, vls


def _pack4(x):
    """[512, N] -> [128, 4N] with the four 128-row blocks side by side."""
    n = x.shape[1]
    return x.reshape(4, 128, n).transpose(1, 0, 2).reshape(128, 4 * n)


def make_in_maps(queries, keys, values, valid_lens, W_q, W_k, W_v, W_o,
                 order, kts):
    bf = ml_dtypes.bfloat16
    queries = np.asarray(queries, np.float32)
    keys = np.asarray(keys, np.float32)
    values = np.asarray(values, np.float32)
    W_q = np.asarray(W_q, np.float32)
    W_k = np.asarray(W_k, np.float32)
    W_v = np.asarray(W_v, np.float32)
    W_o = np.asarray(W_o, np.float32)
    in_maps = []
    for c in range(N_CORES):
        hp, qp = c % 4, c // 4
        fsl = slice(hp * 128, (hp + 1) * 128)
        qsl = slice(qp * QW, (qp + 1) * QW)
        xq = np.concatenate([_pack4(queries[b, qsl, :].T) for b in order],
                            axis=1)
        xk = np.concatenate([_pack4(keys[b, :kts[i], :].T)
                             for i, b in enumerate(order)], axis=1)
        xv = np.concatenate([_pack4(values[b, :kts[i], :].T)
                             for i, b in enumerate(order)], axis=1)
        in_maps.append({
            "xq_t": np.ascontiguousarray(xq).astype(bf),
            "xk_t": np.ascontiguousarray(xk).astype(bf),
            "xv_t": np.ascontiguousarray(xv).astype(bf),
            "wq_t": np.ascontiguousarray(_pack4((W_q[fsl, :] / 8.0).T)).astype(bf),
            "wk_t": np.ascontiguousarray(_pack4(W_k[fsl, :].T)).astype(bf),
            "wv_t": np.ascontiguousarray(_pack4(W_v[fsl, :].T)).astype(bf),
            "wo_t": np.ascontiguousarray(W_o[:, fsl].T).astype(bf),
        })
    return in_maps


def assemble(results, order):
    out = np.empty((B, Q, D), np.float32)
    for qp in range(2):
        for i, b in enumerate(order):
            acc = np.zeros((D, QW), np.float32)
            for hp in range(4):
                acc += np.asarray(results[qp * 4 + hp]["y_t"]
                                  [:, i * QW:(i + 1) * QW], np.float32)
            out[b, qp * QW:(qp + 1) * QW, :] = acc.T
    return out


def kernel(queries, keys, values, valid_lens, W_q, W_k, W_v, W_o):
    order, kts, vls = plan(valid_lens)
    nc = build_nc(kts, vls)
    in_maps = make_in_maps(queries, keys, values, valid_lens,
                           W_q, W_k, W_v, W_o, order, kts)
    res = run_bass_kernel_spmd(nc, in_maps, list(range(N_CORES))).results
    return assemble(res, order)
